# revision 12
# baseline (speedup 1.0000x reference)
"""Trainium2 Bass kernel for BlockFFTDirectPrior.

Computes out = irfft(einsum('bjn,ijn->bin', rfft(x_blocks), conj(W)))
reshaped to [B, 4096], for x [4096, 4096] f32, W [16, 16, 129] complex
(block size 256).

Strategy: data-parallel over the batch axis across 8 NeuronCores (512 rows
each). All operands bf16 with fp32 PSUM accumulation. Three PE stages:

  F: real DFT as matmul (contract t, K=2x128 chunks)  -> X  [n, (c,j,b)]
     cos half row n = Re X[n]; sin half row 0 = the (real) Nyquist bin,
     rows p>=1 = Im X[p].
  E: per-frequency 16x16 complex mixing packed as 4 frequencies x the
     2x2 (re,im) block structure per matmul: K = (c,j,f) = 128, one
     matmul per group g of 4 freqs {32f+g}.            -> Y [(c',i,f), g, b]
  I: real inverse DFT with the data stationary         -> out [b, i*256+m]

The two partition regroups (F->E, E->I) are strided SBUF->SBUF DMAs with
1KB descriptors, split in 8 pieces each across the two HWDGE rings so the
downstream stage can chase piece completion.  The schedule is built so the
16 SDMA engines (the binding resource at ~25 MB of port traffic) never
idle: x-load pieces are chased by F, regroup1 pieces are issued per
F j-quad, stage I chases regroup2 i-quad pieces, and output stores stream
per (row-block, i-half).  Keep-warm dummy matmuls tied to regroup pieces
hold the PE HAM clock gate at 8/8 through the DMA-paced windows.
"""

import os
import numpy as np
from contextlib import ExitStack

import ml_dtypes

import concourse.bass as bass
import concourse.tile as tile
from concourse import bacc, mybir
from concourse.bass_utils import run_bass_kernel_spmd

NCORES = 8
B_FULL, D_IN, D_OUT, BS = 4096, 4096, 4096, 256
BC = B_FULL // NCORES          # 512 batch rows per core
KIN = KOUT = 16
F32 = mybir.dt.float32
BF16 = mybir.dt.bfloat16
NPBF16 = ml_dtypes.bfloat16

_CACHE = {}
LAST_RESULTS = None            # BassKernelResults of the most recent run


def _build_consts(W_real, W_imag):
    """Constant matrices in the exact SBUF layouts the kernel reads (bf16).

    Frequency slot assignment is the identity: DFT-output row r holds
    frequency r; E-group g covers frequencies {32f+g : f=0..3}; the
    IDFT row n corresponds to frequency n.  Partition orders:
      gg rows p = 64c + 4j + f      (E moving operand / wpk rows)
      yy rows q = 64c' + 4i + f     (E output / wpk cols)
      yh rows n = 32f + g = freq    (I stationary / dmat rows)
    """
    t = np.arange(BS)
    n0 = np.arange(128)
    ang = 2.0 * np.pi / BS

    C0 = np.cos(ang * np.outer(t, n0))              # [256, 128] cos rows
    C1 = np.empty((BS, 128))
    C1[:, 0] = np.cos(np.pi * t)                    # Nyquist (real) row
    p = np.arange(1, 128)
    C1[:, 1:] = -np.sin(ang * np.outer(t, p))
    cfs = np.empty((128, 2, 2, 128), dtype=np.float32)
    for c, M in ((0, C0), (1, C1)):
        for tc in range(2):
            cfs[:, c, tc, :] = M[tc * 128:(tc + 1) * 128, :]
    cfs = cfs.astype(NPBF16)

    # wpk[64c+4j+f, g, 64c'+4i+f] = S[(c,j),(c',i)] for freq n = 32f+g
    wpk = np.zeros((128, 32, 128), dtype=np.float32)
    jj = np.arange(KIN)[:, None]
    ii = np.arange(KOUT)[None, :]
    for g in range(32):
        for f in range(4):
            n = 32 * f + g
            if n == 0:
                blocks = (
                    (0, 0, W_real[:, :, 0]),        # DC (W_imag[...,0] == 0)
                    (1, 1, W_real[:, :, 128]),      # Nyquist channel
                )
            else:
                blocks = (
                    (0, 0, W_real[:, :, n]),
                    (1, 0, W_imag[:, :, n]),
                    (0, 1, -W_imag[:, :, n]),
                    (1, 1, W_real[:, :, n]),
                )
            for c, cp, M in blocks:
                # S[(c,j),(c',i)] = M[i, j]
                wpk[64 * c + 4 * jj + f, g, 64 * cp + 4 * ii + f] = M.T[jj, ii]
    wpk = wpk.astype(NPBF16)

    m = np.arange(BS)
    nn = np.arange(1, 128)
    D0 = np.empty((128, BS))
    D0[0] = 1.0 / BS
    D0[1:] = (2.0 / BS) * np.cos(ang * np.outer(nn, m))
    D1 = np.empty((128, BS))
    D1[0] = ((-1.0) ** m) / BS                      # Nyquist IDFT row
    D1[1:] = -(2.0 / BS) * np.sin(ang * np.outer(nn, m))
    dmat = np.stack([D0, D1], axis=1).astype(NPBF16)  # [128, 2, 256]
    return {"cfs": cfs, "wpk": wpk, "dmat": dmat}


def _build_program():
    nc = bacc.Bacc(
        "TRN2", target_bir_lowering=False, debug=False, num_devices=NCORES
    )
    # x pre-transposed on the host: x_d[tl, j, tc, b] = x[b, j*256+tc*128+tl]
    x_d = nc.dram_tensor("x", [128, KIN, 2, BC], BF16, kind="ExternalInput").ap()
    cfs_d = nc.dram_tensor("cfs", [128, 2, 2, 128], BF16, kind="ExternalInput").ap()
    wpk_d = nc.dram_tensor("wpk", [128, 32, 128], BF16, kind="ExternalInput").ap()
    dmat_d = nc.dram_tensor("dmat", [128, 2, 256], BF16, kind="ExternalInput").ap()
    out_d = nc.dram_tensor("out", [BC, D_OUT], BF16, kind="ExternalOutput").ap()

    cp_state = [0]

    with tile.TileContext(nc) as tc, ExitStack() as ctx:
        def copy(dst, src):
            # alternate PSUM->SBUF copies between DVE and ACT
            if cp_state[0] % 2 == 0:
                nc.vector.tensor_copy(dst, src)
            else:
                nc.scalar.copy(dst, src)
            cp_state[0] += 1

        consts = ctx.enter_context(tc.tile_pool(name="consts", bufs=1))
        big = ctx.enter_context(tc.tile_pool(name="big", bufs=1))
        stg = ctx.enter_context(tc.tile_pool(name="stg", bufs=4))
        ps = ctx.enter_context(tc.tile_pool(name="ps", bufs=3, space="PSUM"))
        psn = ctx.enter_context(tc.tile_pool(name="psn", bufs=2, space="PSUM"))

        cfs = consts.tile([128, 2, 2, 128], BF16)
        wpk = consts.tile([128, 32, 128], BF16)
        dmat = consts.tile([128, 2, 256], BF16)

        xt = big.tile([128, KIN, 2, BC], BF16)   # (tl, j, tc, b)
        xf = big.tile([128, 2, KIN, BC], BF16)   # (n,  c, j,  b)
        gg = big.tile([128, 32, BC], BF16)       # ((c,j,f), g, b)
        yy = big.tile([128, 32, BC], BF16)       # ((c',i,f), g, b)
        yh = big.tile([128, 2, KOUT, BC], BF16)  # (n, c', i, b)

        # ---- load: consts + x pieces all on the sync (SP) HWDGE ring so
        # the x pieces complete in order and F can chase them.  wpk/dmat are
        # issued mid-F (after the first regroup1 pieces) so they don't
        # delay either the x stream or the regroup1 descriptors.
        nc.sync.dma_start(cfs[:], cfs_d)
        for k in range(4):
            nc.sync.dma_start(
                xt[:, 4 * k:4 * k + 4, :, :], x_d[:, 4 * k:4 * k + 4, :, :]
            )

        # ---- PE warmup while the first x piece streams in
        cfs_flat = cfs[:].rearrange("p c t r -> p (c t r)")
        for w in range(8):
            pw = psn.tile([128, 512], F32, tag="psn")
            nc.tensor.matmul(
                pw[:], cfs[:, 0, 0, :], cfs_flat, start=True, stop=True
            )

        # regroup1: gg[(c,j,f), g, b] = xf[32f+g, c, j, b].  One piece per
        # (c, j): dst = 4 consecutive partitions (iteration f, g, b), src =
        # all 128 partitions (iteration n=32f+g, b) — both partition-outer.
        # c=0 pieces ride the SP HWDGE ring, c=1 the gpsimd SWDGE ring; ACT
        # stays DMA-free here so the PSUM-drain copies never queue behind a
        # descriptor-generation stall.
        def r1_piece(c, j):
            eng = nc.sync if c == 0 else nc.gpsimd
            p0 = 64 * c + 4 * j
            eng.dma_start(out=gg[p0:p0 + 4, :, :], in_=xf[:, c, j, :])

        # ---- stage F chases the x pieces; regroup1 pieces chase F.
        for jp in range(8):
            for c in range(2):
                pf = ps.tile([128, 2, BC], F32, tag="ps")
                for jj in range(2):
                    j = 2 * jp + jj
                    nc.tensor.matmul(
                        pf[:, jj, :], cfs[:, c, 0, :], xt[:, j, 0, :],
                        start=True, stop=False,
                    )
                    nc.tensor.matmul(
                        pf[:, jj, :], cfs[:, c, 1, :], xt[:, j, 1, :],
                        start=False, stop=True,
                    )
                copy(xf[:, c, 2 * jp:2 * jp + 2, :], pf[:])
            for c in range(2):
                r1_piece(c, 2 * jp)
                r1_piece(c, 2 * jp + 1)
            if jp == 1:
                nc.sync.dma_start(wpk[:], wpk_d)
                nc.sync.dma_start(dmat[:], dmat_d)

        # ---- keep-warm dummies tied to regroup1 pieces: one matmul reading
        # a slice of each piece keeps the PE HAM gate open while the
        # SBUF->SBUF regroup streams.  Matmul base partition must be one of
        # {0, 32, 64}, so pick (base, K) covering each piece's rows.
        R1_SPAN = {
            (0, 0): (0, 16), (0, 1): (0, 32), (0, 2): (32, 16), (0, 3): (32, 32),
            (1, 0): (64, 16), (1, 1): (64, 32), (1, 2): (64, 48), (1, 3): (64, 64),
        }
        for q in range(4):
            for c in range(2):
                base, kk = R1_SPAN[(c, q)]
                for gslice in (2 * q, 2 * q + 1):
                    pd = psn.tile([128, 512], F32, tag="psn")
                    nc.tensor.matmul(
                        pd[:],
                        cfs[base:base + kk, 0, 0, :],
                        gg[base:base + kk, gslice, :],
                        start=True, stop=True,
                    )

        # ---- stage E: one matmul per 4-frequency group (K = (c,j,f) = 128)
        for gp in range(16):
            pe = ps.tile([128, 2, BC], F32, tag="ps")
            for gg_i in range(2):
                g = 2 * gp + gg_i
                nc.tensor.matmul(
                    pe[:, gg_i, :], wpk[:, g, :], gg[:, g, :],
                    start=True, stop=True,
                )
            copy(yy[:, 2 * gp:2 * gp + 2, :], pe[:])

        # regroup2: yh[32f+g, c', i, b] = yy[(c',i,f), g, b].  One piece per
        # (c', i): src = 4 consecutive partitions (iteration f, g, b), dst =
        # all 128 partitions (iteration n, b).  c=0 on the ACT HWDGE ring
        # (emitted after its last E copy, so no head-of-line risk), c=1 on
        # the gpsimd SWDGE ring.
        def r2_piece(c, i):
            eng = nc.scalar if c == 0 else nc.gpsimd
            p0 = 64 * c + 4 * i
            eng.dma_start(out=yh[:, c, i, :], in_=yy[p0:p0 + 4, :, :])

        for i in range(KOUT):
            r2_piece(0, i)
            r2_piece(1, i)

        # keep-warm dummies on the first regroup2 pieces (read free-dim
        # slice i = 4q of the c'=0 piece q from a base-0 partition range)
        for q in range(2):
            pd = psn.tile([128, 512], F32, tag="psn")
            nc.tensor.matmul(
                pd[:],
                dmat[0:16, 0, 0:128],
                yh[0:16, 0, 4 * q, :],
                start=True, stop=True,
            )

        # ---- stage I: inverse DFT, data stationary; chases regroup2
        # i-quad pieces; stores stream per (row-block, i-half).
        for ih in range(2):
            ot = [
                stg.tile([128, 2048], BF16, tag="stg", name=f"ot{ih}_{bs}")
                for bs in range(4)
            ]
            for iqh in range(2):
                iq = 2 * ih + iqh
                for bs in range(4):
                    po = ps.tile([128, 4, 256], F32, tag="ps")
                    for k in range(4):
                        i = 4 * iq + k
                        bsl = yh[:, :, i, 128 * bs:128 * (bs + 1)]
                        nc.tensor.matmul(
                            po[:, k, :], bsl[:, 0, :], dmat[:, 0, :],
                            start=True, stop=False,
                        )
                        nc.tensor.matmul(
                            po[:, k, :], bsl[:, 1, :], dmat[:, 1, :],
                            start=False, stop=True,
                        )
                    copy(ot[bs][:, 1024 * iqh:1024 * (iqh + 1)], po[:])
            for bs in range(4):
                nc.sync.dma_start(
                    out_d[128 * bs:128 * (bs + 1), 2048 * ih:2048 * (ih + 1)],
                    ot[bs][:],
                )

    nc.compile()
    return nc


def _get_program():
    if "nc" not in _CACHE:
        _CACHE["nc"] = _build_program()
    return _CACHE["nc"]


def _install_ntff_hook():
    """Provide antenv.axon_hooks (absent in this image) so that
    run_bass_kernel_spmd(trace=True) can capture NTFF profiles through the
    axon client library."""
    import sys
    import types
    import ctypes
    import contextlib

    if "antenv.axon_hooks" in sys.modules:
        return
    try:
        lib = ctypes.CDLL("/opt/axon/libaxon_pjrt.so")
    except OSError:
        return
    if not hasattr(lib, "axon_start_nrt_profile"):
        return
    lib.axon_start_nrt_profile.argtypes = [
        ctypes.POINTER(ctypes.c_int64),
        ctypes.c_size_t,
    ]
    lib.axon_start_nrt_profile.restype = ctypes.c_int64
    lib.axon_stop_nrt_profile.argtypes = [ctypes.c_char_p]
    lib.axon_stop_nrt_profile.restype = ctypes.c_int64

    @contextlib.contextmanager
    def _hook(output_dir, device_ids):
        import jax

        jax.devices()
        if device_ids:
            ids = (ctypes.c_int64 * len(device_ids))(*device_ids)
            rc = lib.axon_start_nrt_profile(ids, len(device_ids))
        else:
            rc = lib.axon_start_nrt_profile(None, 0)
        if rc != 0:
            raise RuntimeError(f"axon_start_nrt_profile rc={rc}")
        try:
            yield
        finally:
            n = lib.axon_stop_nrt_profile(str(output_dir).encode())
            print(f"ntff profile: {n} file(s) -> {output_dir}")

    mod = types.ModuleType("antenv.axon_hooks")
    state = {"hook": _hook}
    mod.get_axon_ntff_profile_hook = lambda: state["hook"]
    mod.set_axon_ntff_profile_hook = lambda h: state.update(hook=h)
    sys.modules["antenv.axon_hooks"] = mod
    import antenv

    antenv.axon_hooks = mod


def kernel(x, W_real, W_imag, block_size, out_features):
    global LAST_RESULTS
    x = np.asarray(x, dtype=np.float32)
    Wr = np.asarray(W_real, dtype=np.float32)
    Wi = np.asarray(W_imag, dtype=np.float32)
    assert int(block_size) == BS and int(out_features) == D_OUT
    assert x.shape == (B_FULL, D_IN) and Wr.shape == (KOUT, KIN, 129)

    nc = _get_program()
    consts = _build_consts(Wr, Wi)
    core_ids = list(range(NCORES))
    # host-side: cast to bf16 and transpose each core's shard into
    # xt[tl, j, tc, b] = x[b, j*256 + tc*128 + tl]
    xb = x.astype(NPBF16).reshape(NCORES, BC, KIN, 2, 128)
    in_maps = [
        {"x": np.ascontiguousarray(xb[c].transpose(3, 1, 2, 0)), **consts}
        for c in core_ids
    ]
    trace = bool(int(os.environ.get("KERNEL_TRACE", "0")))
    if trace:
        _install_ntff_hook()
    res = run_bass_kernel_spmd(nc, in_maps, core_ids, trace=trace)
    LAST_RESULTS = res
    out = np.concatenate([res.results[c]["out"] for c in core_ids], axis=0)
    return np.ascontiguousarray(out.astype(np.float32))


# revision 17
# speedup vs baseline: 1.2963x; 1.2963x over previous
"""Trainium2 Bass kernel for BlockFFTDirectPrior.

Computes out = irfft(einsum('bjn,ijn->bin', rfft(x_blocks), conj(W)))
reshaped to [B, 4096], for x [4096, 4096] f32, W [16, 16, 129] complex
(block size 256).

Strategy: data-parallel over the batch axis across 8 NeuronCores (512 rows
each). All operands bf16 with fp32 PSUM accumulation. Three PE stages:

  F: real DFT as matmul (contract t, K=2x128 chunks)  -> X  [n, (c,j,b)]
     cos half row n = Re X[n]; sin half row 0 = the (real) Nyquist bin,
     rows p>=1 = Im X[p].
  E: per-frequency 16x16 complex mixing packed as 4 frequencies x the
     2x2 (re,im) block structure per matmul: K = (c,j,f) = 128, one
     matmul per group g of 4 freqs {32f+g}.            -> Y [(c',i,f), g, b]
  I: real inverse DFT with the data stationary         -> out [b, i*256+m]

The two partition regroups (F->E, E->I) are strided SBUF->SBUF DMAs with
1KB descriptors, split in 8 pieces each across the two HWDGE rings so the
downstream stage can chase piece completion.  The schedule is built so the
16 SDMA engines (the binding resource at ~25 MB of port traffic) never
idle: x-load pieces are chased by F, regroup1 pieces are issued per
F j-quad, stage I chases regroup2 i-quad pieces, and output stores stream
per (row-block, i-half).  Keep-warm dummy matmuls tied to regroup pieces
hold the PE HAM clock gate at 8/8 through the DMA-paced windows.
"""

import os
import numpy as np
from contextlib import ExitStack

import ml_dtypes

import concourse.bass as bass
import concourse.tile as tile
from concourse import bacc, mybir
from concourse.bass_utils import run_bass_kernel_spmd

NCORES = 8
B_FULL, D_IN, D_OUT, BS = 4096, 4096, 4096, 256
BC = B_FULL // NCORES          # 512 batch rows per core
KIN = KOUT = 16
F32 = mybir.dt.float32
BF16 = mybir.dt.bfloat16
NPBF16 = ml_dtypes.bfloat16

_CACHE = {}
LAST_RESULTS = None            # BassKernelResults of the most recent run


def _build_consts(W_real, W_imag):
    """Constant matrices in the exact SBUF layouts the kernel reads (bf16).

    Frequency slot assignment is the identity: DFT-output row r holds
    frequency r; E-group g covers frequencies {32f+g : f=0..3}; the
    IDFT row n corresponds to frequency n.  Partition orders:
      gg rows p = 64c + 4j + f      (E moving operand / wpk rows)
      yy rows q = 64c' + 4i + f     (E output / wpk cols)
      yh rows n = 32f + g = freq    (I stationary / dmat rows)
    """
    t = np.arange(BS)
    n0 = np.arange(128)
    ang = 2.0 * np.pi / BS

    C0 = np.cos(ang * np.outer(t, n0))              # [256, 128] cos rows
    C1 = np.empty((BS, 128))
    C1[:, 0] = np.cos(np.pi * t)                    # Nyquist (real) row
    p = np.arange(1, 128)
    C1[:, 1:] = -np.sin(ang * np.outer(t, p))
    cfs = np.empty((128, 2, 2, 128), dtype=np.float32)
    for c, M in ((0, C0), (1, C1)):
        for tc in range(2):
            cfs[:, c, tc, :] = M[tc * 128:(tc + 1) * 128, :]
    cfs = cfs.astype(NPBF16)

    # wpk[P(c,j,f), g, P(c',i,f)] = S[(c,j),(c',i)] for freq n = 32f+g.
    # P interleaves f with stride 4 so each regroup piece's 4 partitions
    # {base, base+4, base+8, base+12} land on 4 different SBUF AXI ports.
    def P(c, k, f):
        return 64 * c + 16 * (k // 4) + 4 * f + (k % 4)

    wpk = np.zeros((128, 32, 128), dtype=np.float32)
    jj = np.arange(KIN)[:, None]
    ii = np.arange(KOUT)[None, :]
    for g in range(32):
        for f in range(4):
            n = 32 * f + g
            if n == 0:
                blocks = (
                    (0, 0, W_real[:, :, 0]),        # DC (W_imag[...,0] == 0)
                    (1, 1, W_real[:, :, 128]),      # Nyquist channel
                )
            else:
                blocks = (
                    (0, 0, W_real[:, :, n]),
                    (1, 0, W_imag[:, :, n]),
                    (0, 1, -W_imag[:, :, n]),
                    (1, 1, W_real[:, :, n]),
                )
            for c, cp, M in blocks:
                # S[(c,j),(c',i)] = M[i, j]
                wpk[P(c, jj, f), g, P(cp, ii, f)] = M.T[jj, ii]
    wpk = wpk.astype(NPBF16)

    m = np.arange(BS)
    nn = np.arange(1, 128)
    D0 = np.empty((128, BS))
    D0[0] = 1.0 / BS
    D0[1:] = (2.0 / BS) * np.cos(ang * np.outer(nn, m))
    D1 = np.empty((128, BS))
    D1[0] = ((-1.0) ** m) / BS                      # Nyquist IDFT row
    D1[1:] = -(2.0 / BS) * np.sin(ang * np.outer(nn, m))
    dmat = np.stack([D0, D1], axis=1).astype(NPBF16)  # [128, 2, 256]
    return {"cfs": cfs, "wpk": wpk, "dmat": dmat}


def _build_program():
    nc = bacc.Bacc(
        "TRN2", target_bir_lowering=False, debug=False, num_devices=NCORES
    )
    # x pre-transposed on the host: x_d[tl, j, tc, b] = x[b, j*256+tc*128+tl]
    x_d = nc.dram_tensor("x", [128, KIN, 2, BC], BF16, kind="ExternalInput").ap()
    cfs_d = nc.dram_tensor("cfs", [128, 2, 2, 128], BF16, kind="ExternalInput").ap()
    wpk_d = nc.dram_tensor("wpk", [128, 32, 128], BF16, kind="ExternalInput").ap()
    dmat_d = nc.dram_tensor("dmat", [128, 2, 256], BF16, kind="ExternalInput").ap()
    out_d = nc.dram_tensor("out", [BC, D_OUT], BF16, kind="ExternalOutput").ap()

    cp_state = [0]

    with tile.TileContext(nc) as tc, ExitStack() as ctx:
        def copy(dst, src):
            # alternate PSUM->SBUF copies between DVE and ACT
            if cp_state[0] % 2 == 0:
                nc.vector.tensor_copy(dst, src)
            else:
                nc.scalar.copy(dst, src)
            cp_state[0] += 1

        consts = ctx.enter_context(tc.tile_pool(name="consts", bufs=1))
        big = ctx.enter_context(tc.tile_pool(name="big", bufs=1))
        stg = ctx.enter_context(tc.tile_pool(name="stg", bufs=4))
        ps = ctx.enter_context(tc.tile_pool(name="ps", bufs=3, space="PSUM"))
        psn = ctx.enter_context(tc.tile_pool(name="psn", bufs=2, space="PSUM"))

        cfs = consts.tile([128, 2, 2, 128], BF16)
        wpk = consts.tile([128, 32, 128], BF16)
        dmat = consts.tile([128, 2, 256], BF16)

        xt = big.tile([128, KIN, 2, BC], BF16)   # (tl, j, tc, b)
        xf = big.tile([128, 2, KIN, BC], BF16)   # (n,  c, j,  b)
        gg = big.tile([128, 32, BC], BF16)       # ((c,j,f), g, b)
        yy = big.tile([128, 32, BC], BF16)       # ((c',i,f), g, b)
        yh = big.tile([128, 2, KOUT, BC], BF16)  # (n, c', i, b)

        # ---- load: consts + x pieces all on the sync (SP) HWDGE ring so
        # the x pieces complete in order and F can chase them.  wpk/dmat are
        # issued mid-F (after the first regroup1 pieces) so they don't
        # delay either the x stream or the regroup1 descriptors.
        nc.sync.dma_start(cfs[:], cfs_d)
        for k in range(4):
            nc.sync.dma_start(
                xt[:, 4 * k:4 * k + 4, :, :], x_d[:, 4 * k:4 * k + 4, :, :]
            )

        # ---- PE warmup while the first x piece streams in
        cfs_flat = cfs[:].rearrange("p c t r -> p (c t r)")
        for w in range(8):
            pw = psn.tile([128, 512], F32, tag="psn")
            nc.tensor.matmul(
                pw[:], cfs[:, 0, 0, :], cfs_flat, start=True, stop=True
            )

        # regroup1: gg[P(c,j,f), g, b] = xf[32f+g, c, j, b].  One piece per
        # (c, j): dst = 4 partitions {base+4f} spanning 4 SBUF ports
        # (iteration f, g, b), src = all 128 partitions (iteration n, b) —
        # both partition-outer.  c=0 pieces ride the SP HWDGE ring, c=1 the
        # gpsimd SWDGE ring; ACT stays DMA-free here so the PSUM-drain
        # copies never queue behind a descriptor-generation stall.
        def r1_piece(c, j):
            eng = nc.sync if c == 0 else nc.gpsimd
            p0 = 64 * c + 16 * (j // 4) + (j % 4)
            eng.dma_start(out=gg[p0:p0 + 13:4, :, :], in_=xf[:, c, j, :])

        # ---- stage F chases the x pieces; regroup1 pieces chase F.
        for jp in range(8):
            for c in range(2):
                pf = ps.tile([128, 2, BC], F32, tag="ps")
                for jj in range(2):
                    j = 2 * jp + jj
                    nc.tensor.matmul(
                        pf[:, jj, :], cfs[:, c, 0, :], xt[:, j, 0, :],
                        start=True, stop=False,
                    )
                    nc.tensor.matmul(
                        pf[:, jj, :], cfs[:, c, 1, :], xt[:, j, 1, :],
                        start=False, stop=True,
                    )
                copy(xf[:, c, 2 * jp:2 * jp + 2, :], pf[:])
            for c in range(2):
                r1_piece(c, 2 * jp)
                r1_piece(c, 2 * jp + 1)
            if jp == 1:
                nc.sync.dma_start(wpk[:], wpk_d)
                nc.sync.dma_start(dmat[:], dmat_d)

        # ---- keep-warm dummies tied to regroup1 pieces: one matmul reading
        # a slice of each piece keeps the PE HAM gate open while the
        # SBUF->SBUF regroup streams.  Matmul base partition must be one of
        # {0, 32, 64}, so pick (base, K) covering each piece's rows.
        R1_SPAN = {
            (0, 0): (0, 16), (0, 1): (0, 32), (0, 2): (32, 16), (0, 3): (32, 32),
            (1, 0): (64, 16), (1, 1): (64, 32), (1, 2): (64, 48), (1, 3): (64, 64),
        }
        for q in range(4):
            for c in range(2):
                base, kk = R1_SPAN[(c, q)]
                for gslice in (2 * q, 2 * q + 1):
                    pd = psn.tile([128, 512], F32, tag="psn")
                    nc.tensor.matmul(
                        pd[:],
                        cfs[base:base + kk, 0, 0, :],
                        gg[base:base + kk, gslice, :],
                        start=True, stop=True,
                    )

        # ---- stage E: one matmul per 4-frequency group (K = (c,j,f) = 128)
        for gp in range(16):
            pe = ps.tile([128, 2, BC], F32, tag="ps")
            for gg_i in range(2):
                g = 2 * gp + gg_i
                nc.tensor.matmul(
                    pe[:, gg_i, :], wpk[:, g, :], gg[:, g, :],
                    start=True, stop=True,
                )
            copy(yy[:, 2 * gp:2 * gp + 2, :], pe[:])

        # regroup2: yh[32f+g, c', i, b] = yy[(c',i,f), g, b].  One piece per
        # (c', i): src = 4 consecutive partitions (iteration f, g, b), dst =
        # all 128 partitions (iteration n, b).  c=0 on the ACT HWDGE ring
        # (emitted after its last E copy, so no head-of-line risk), c=1 on
        # the gpsimd SWDGE ring.
        def r2_piece(c, i):
            eng = nc.scalar if c == 0 else nc.gpsimd
            p0 = 64 * c + 16 * (i // 4) + (i % 4)
            eng.dma_start(out=yh[:, c, i, :], in_=yy[p0:p0 + 13:4, :, :])

        for i in range(KOUT):
            r2_piece(0, i)
            r2_piece(1, i)

        # keep-warm dummies on the first regroup2 pieces (read free-dim
        # slice i = 4q of the c'=0 piece q from a base-0 partition range)
        for q in range(2):
            pd = psn.tile([128, 512], F32, tag="psn")
            nc.tensor.matmul(
                pd[:],
                dmat[0:16, 0, 0:128],
                yh[0:16, 0, 4 * q, :],
                start=True, stop=True,
            )

        # ---- stage I: inverse DFT, data stationary; chases regroup2
        # i-quad pieces; stores stream per (row-block, i-half).
        for ih in range(2):
            ot = [
                stg.tile([128, 2048], BF16, tag="stg", name=f"ot{ih}_{bs}")
                for bs in range(4)
            ]
            for iqh in range(2):
                iq = 2 * ih + iqh
                for bs in range(4):
                    po = ps.tile([128, 4, 256], F32, tag="ps")
                    for k in range(4):
                        i = 4 * iq + k
                        bsl = yh[:, :, i, 128 * bs:128 * (bs + 1)]
                        nc.tensor.matmul(
                            po[:, k, :], bsl[:, 0, :], dmat[:, 0, :],
                            start=True, stop=False,
                        )
                        nc.tensor.matmul(
                            po[:, k, :], bsl[:, 1, :], dmat[:, 1, :],
                            start=False, stop=True,
                        )
                    copy(ot[bs][:, 1024 * iqh:1024 * (iqh + 1)], po[:])
            for bs in range(4):
                nc.gpsimd.dma_start(
                    out_d[128 * bs:128 * (bs + 1), 2048 * ih:2048 * (ih + 1)],
                    ot[bs][:],
                )

    nc.compile()
    return nc


def _get_program():
    if "nc" not in _CACHE:
        _CACHE["nc"] = _build_program()
    return _CACHE["nc"]


def _install_ntff_hook():
    """Provide antenv.axon_hooks (absent in this image) so that
    run_bass_kernel_spmd(trace=True) can capture NTFF profiles through the
    axon client library."""
    import sys
    import types
    import ctypes
    import contextlib

    if "antenv.axon_hooks" in sys.modules:
        return
    try:
        lib = ctypes.CDLL("/opt/axon/libaxon_pjrt.so")
    except OSError:
        return
    if not hasattr(lib, "axon_start_nrt_profile"):
        return
    lib.axon_start_nrt_profile.argtypes = [
        ctypes.POINTER(ctypes.c_int64),
        ctypes.c_size_t,
    ]
    lib.axon_start_nrt_profile.restype = ctypes.c_int64
    lib.axon_stop_nrt_profile.argtypes = [ctypes.c_char_p]
    lib.axon_stop_nrt_profile.restype = ctypes.c_int64

    @contextlib.contextmanager
    def _hook(output_dir, device_ids):
        import jax

        jax.devices()
        if device_ids:
            ids = (ctypes.c_int64 * len(device_ids))(*device_ids)
            rc = lib.axon_start_nrt_profile(ids, len(device_ids))
        else:
            rc = lib.axon_start_nrt_profile(None, 0)
        if rc != 0:
            raise RuntimeError(f"axon_start_nrt_profile rc={rc}")
        try:
            yield
        finally:
            n = lib.axon_stop_nrt_profile(str(output_dir).encode())
            print(f"ntff profile: {n} file(s) -> {output_dir}")

    mod = types.ModuleType("antenv.axon_hooks")
    state = {"hook": _hook}
    mod.get_axon_ntff_profile_hook = lambda: state["hook"]
    mod.set_axon_ntff_profile_hook = lambda h: state.update(hook=h)
    sys.modules["antenv.axon_hooks"] = mod
    import antenv

    antenv.axon_hooks = mod


def kernel(x, W_real, W_imag, block_size, out_features):
    global LAST_RESULTS
    x = np.asarray(x, dtype=np.float32)
    Wr = np.asarray(W_real, dtype=np.float32)
    Wi = np.asarray(W_imag, dtype=np.float32)
    assert int(block_size) == BS and int(out_features) == D_OUT
    assert x.shape == (B_FULL, D_IN) and Wr.shape == (KOUT, KIN, 129)

    nc = _get_program()
    consts = _build_consts(Wr, Wi)
    core_ids = list(range(NCORES))
    # host-side: cast to bf16 and transpose each core's shard into
    # xt[tl, j, tc, b] = x[b, j*256 + tc*128 + tl]
    xb = x.astype(NPBF16).reshape(NCORES, BC, KIN, 2, 128)
    in_maps = [
        {"x": np.ascontiguousarray(xb[c].transpose(3, 1, 2, 0)), **consts}
        for c in core_ids
    ]
    trace = bool(int(os.environ.get("KERNEL_TRACE", "0")))
    if trace:
        _install_ntff_hook()
    res = run_bass_kernel_spmd(nc, in_maps, core_ids, trace=trace)
    LAST_RESULTS = res
    out = np.concatenate([res.results[c]["out"] for c in core_ids], axis=0)
    return np.ascontiguousarray(out.astype(np.float32))


# revision 21
# speedup vs baseline: 1.4561x; 1.1232x over previous
"""Trainium2 Bass kernel for BlockFFTDirectPrior.

Computes out = irfft(einsum('bjn,ijn->bin', rfft(x_blocks), conj(W)))
reshaped to [B, 4096], for x [4096, 4096] f32, W [16, 16, 129] complex
(block size 256).

Strategy: data-parallel over the batch axis across 8 NeuronCores (512 rows
each). All operands bf16 with fp32 PSUM accumulation. Three PE stages:

  F: real DFT as matmul (contract t, K=2x128 chunks)  -> X  [n, (c,j,b)]
     cos half row n = Re X[n]; sin half row 0 = the (real) Nyquist bin,
     rows p>=1 = Im X[p].
  E: per-frequency 16x16 complex mixing packed as 4 frequencies x the
     2x2 (re,im) block structure per matmul: K = (c,j,f) = 128, one
     matmul per group g of 4 freqs {32f+g}.            -> Y [(c',i,f), g, b]
  I: real inverse DFT with the data stationary         -> out [b, i*256+m]

The two partition regroups (F->E, E->I) are strided SBUF->SBUF DMAs with
1KB descriptors, split in 8 pieces each across the two HWDGE rings so the
downstream stage can chase piece completion.  The schedule is built so the
16 SDMA engines (the binding resource at ~25 MB of port traffic) never
idle: x-load pieces are chased by F, regroup1 pieces are issued per
F j-quad, stage I chases regroup2 i-quad pieces, and output stores stream
per (row-block, i-half).  Keep-warm dummy matmuls tied to regroup pieces
hold the PE HAM clock gate at 8/8 through the DMA-paced windows.
"""

import os
import numpy as np
from contextlib import ExitStack

import ml_dtypes

import concourse.bass as bass
import concourse.tile as tile
from concourse import bacc, mybir
from concourse.bass_utils import run_bass_kernel_spmd

NCORES = 8
B_FULL, D_IN, D_OUT, BS = 4096, 4096, 4096, 256
BC = B_FULL // NCORES          # 512 batch rows per core
KIN = KOUT = 16
F32 = mybir.dt.float32
BF16 = mybir.dt.bfloat16
NPBF16 = ml_dtypes.bfloat16

_CACHE = {}
LAST_RESULTS = None            # BassKernelResults of the most recent run


def _build_consts(W_real, W_imag):
    """Constant matrices in the exact SBUF layouts the kernel reads (bf16).

    Frequency slot assignment is the identity: DFT-output row r holds
    frequency r; E-group g covers frequencies {32f+g : f=0..3}; the
    IDFT row n corresponds to frequency n.  Partition orders:
      gg rows p = 64c + 4j + f      (E moving operand / wpk rows)
      yy rows q = 64c' + 4i + f     (E output / wpk cols)
      yh rows n = 32f + g = freq    (I stationary / dmat rows)
    """
    t = np.arange(BS)
    n0 = np.arange(128)
    ang = 2.0 * np.pi / BS

    C0 = np.cos(ang * np.outer(t, n0))              # [256, 128] cos rows
    C1 = np.empty((BS, 128))
    C1[:, 0] = np.cos(np.pi * t)                    # Nyquist (real) row
    p = np.arange(1, 128)
    C1[:, 1:] = -np.sin(ang * np.outer(t, p))
    cfs = np.empty((128, 2, 2, 128), dtype=np.float32)
    for c, M in ((0, C0), (1, C1)):
        for tc in range(2):
            cfs[:, c, tc, :] = M[tc * 128:(tc + 1) * 128, :]
    cfs = cfs.astype(NPBF16)

    # wpk[P(c,j,f), g, P(c',i,f)] = S[(c,j),(c',i)] for freq n = 32f+g.
    # P interleaves f with stride 4 so each regroup piece's 4 partitions
    # {base, base+4, base+8, base+12} land on 4 different SBUF AXI ports.
    def P(c, k, f):
        return 64 * c + 16 * (k // 4) + 4 * f + (k % 4)

    wpk = np.zeros((128, 32, 128), dtype=np.float32)
    jj = np.arange(KIN)[:, None]
    ii = np.arange(KOUT)[None, :]
    for g in range(32):
        for f in range(4):
            n = 32 * f + g
            if n == 0:
                blocks = (
                    (0, 0, W_real[:, :, 0]),        # DC (W_imag[...,0] == 0)
                    (1, 1, W_real[:, :, 128]),      # Nyquist channel
                )
            else:
                blocks = (
                    (0, 0, W_real[:, :, n]),
                    (1, 0, W_imag[:, :, n]),
                    (0, 1, -W_imag[:, :, n]),
                    (1, 1, W_real[:, :, n]),
                )
            for c, cp, M in blocks:
                # S[(c,j),(c',i)] = M[i, j]
                wpk[P(c, jj, f), g, P(cp, ii, f)] = M.T[jj, ii]
    wpk = wpk.astype(NPBF16)

    m = np.arange(BS)
    nn = np.arange(1, 128)
    D0 = np.empty((128, BS))
    D0[0] = 1.0 / BS
    D0[1:] = (2.0 / BS) * np.cos(ang * np.outer(nn, m))
    D1 = np.empty((128, BS))
    D1[0] = ((-1.0) ** m) / BS                      # Nyquist IDFT row
    D1[1:] = -(2.0 / BS) * np.sin(ang * np.outer(nn, m))
    dmat = np.stack([D0, D1], axis=1).astype(NPBF16)  # [128, 2, 256]
    return {"cfs": cfs, "wpk": wpk, "dmat": dmat}


def _build_program():
    nc = bacc.Bacc(
        "TRN2", target_bir_lowering=False, debug=False, num_devices=NCORES
    )
    # x pre-transposed on the host: x_d[tl, j, tc, b] = x[b, j*256+tc*128+tl]
    x_d = nc.dram_tensor("x", [128, KIN, 2, BC], BF16, kind="ExternalInput").ap()
    cfs_d = nc.dram_tensor("cfs", [128, 2, 2, 128], BF16, kind="ExternalInput").ap()
    wpk_d = nc.dram_tensor("wpk", [128, 32, 128], BF16, kind="ExternalInput").ap()
    dmat_d = nc.dram_tensor("dmat", [128, 2, 256], BF16, kind="ExternalInput").ap()
    out_d = nc.dram_tensor("out", [BC, D_OUT], BF16, kind="ExternalOutput").ap()

    cp_state = [0]

    with tile.TileContext(nc) as tc, ExitStack() as ctx:
        def copy(dst, src):
            # alternate PSUM->SBUF copies between DVE and ACT
            if cp_state[0] % 2 == 0:
                nc.vector.tensor_copy(dst, src)
            else:
                nc.scalar.copy(dst, src)
            cp_state[0] += 1

        consts = ctx.enter_context(tc.tile_pool(name="consts", bufs=1))
        big = ctx.enter_context(tc.tile_pool(name="big", bufs=1))
        stg = ctx.enter_context(tc.tile_pool(name="stg", bufs=4))
        ps = ctx.enter_context(tc.tile_pool(name="ps", bufs=3, space="PSUM"))
        psn = ctx.enter_context(tc.tile_pool(name="psn", bufs=2, space="PSUM"))

        cfs = consts.tile([128, 2, 2, 128], BF16)
        wpk = consts.tile([128, 32, 128], BF16)
        dmat = consts.tile([128, 2, 256], BF16)

        xt = big.tile([128, KIN, 2, BC], BF16)   # (tl, j, tc, b)
        xf = big.tile([128, 2, KIN, BC], BF16)   # (n,  c, j,  b)
        gg = big.tile([128, 32, BC], BF16)       # ((c,j,f), g, b)
        yy = big.tile([128, 32, BC], BF16)       # ((c',i,f), g, b)
        yh = big.tile([128, 2, KOUT, BC], BF16)  # (n, c', i, b)

        # ---- load: consts + x pieces all on the sync (SP) HWDGE ring so
        # the x pieces complete in order and F can chase them.  wpk/dmat are
        # issued mid-F (after the first regroup1 pieces) so they don't
        # delay either the x stream or the regroup1 descriptors.
        nc.sync.dma_start(cfs[:], cfs_d)
        for k in range(4):
            nc.sync.dma_start(
                xt[:, 4 * k:4 * k + 4, :, :], x_d[:, 4 * k:4 * k + 4, :, :]
            )

        # ---- PE warmup while the first x piece streams in
        cfs_flat = cfs[:].rearrange("p c t r -> p (c t r)")
        for w in range(8):
            pw = psn.tile([128, 512], F32, tag="psn")
            nc.tensor.matmul(
                pw[:], cfs[:, 0, 0, :], cfs_flat, start=True, stop=True
            )

        # regroup1: gg[P(c,j,f), g, b] = xf[32f+g, c, j, b].  One piece per
        # (c, j): dst = 4 partitions {base+4f} spanning 4 SBUF ports
        # (iteration f, g, b), src = all 128 partitions (iteration n, b) —
        # both partition-outer.  c=0 pieces ride the SP HWDGE ring, c=1 the
        # gpsimd SWDGE ring; ACT stays DMA-free here so the PSUM-drain
        # copies never queue behind a descriptor-generation stall.
        def r1_piece(c, j):
            # SWDGE (Q7) emits these ~3x faster than a backpressured HWDGE
            # ring, so it takes 3/4 of the pieces.
            eng = nc.sync if (c == 0 and j % 2 == 0) else nc.gpsimd
            p0 = 64 * c + 16 * (j // 4) + (j % 4)
            eng.dma_start(out=gg[p0:p0 + 13:4, :, :], in_=xf[:, c, j, :])

        # ---- stage F chases the x pieces; regroup1 pieces chase F.
        for jp in range(8):
            for c in range(2):
                pf = ps.tile([128, 2, BC], F32, tag="ps")
                for jj in range(2):
                    j = 2 * jp + jj
                    nc.tensor.matmul(
                        pf[:, jj, :], cfs[:, c, 0, :], xt[:, j, 0, :],
                        start=True, stop=False,
                    )
                    nc.tensor.matmul(
                        pf[:, jj, :], cfs[:, c, 1, :], xt[:, j, 1, :],
                        start=False, stop=True,
                    )
                copy(xf[:, c, 2 * jp:2 * jp + 2, :], pf[:])
            for c in range(2):
                r1_piece(c, 2 * jp)
                r1_piece(c, 2 * jp + 1)
            if jp == 1:
                nc.sync.dma_start(wpk[:], wpk_d)
                nc.sync.dma_start(dmat[:], dmat_d)

        # ---- keep-warm dummies tied to regroup1 pieces: one matmul reading
        # a slice of each piece keeps the PE HAM gate open while the
        # SBUF->SBUF regroup streams.  Matmul base partition must be one of
        # {0, 32, 64}, so pick (base, K) covering each piece's rows.
        R1_SPAN = {
            (0, 0): (0, 16), (0, 1): (0, 32), (0, 2): (32, 16), (0, 3): (32, 32),
            (1, 0): (64, 16), (1, 1): (64, 32), (1, 2): (64, 48), (1, 3): (64, 64),
        }
        for q in range(4):
            for c in range(2):
                base, kk = R1_SPAN[(c, q)]
                for gslice in (4 * q, 4 * q + 1, 4 * q + 2, 4 * q + 3):
                    pd = psn.tile([128, 512], F32, tag="psn")
                    nc.tensor.matmul(
                        pd[:],
                        cfs[base:base + kk, 0, 0, :],
                        gg[base:base + kk, gslice, :],
                        start=True, stop=True,
                    )

        # ---- stage E: one matmul per 4-frequency group (K = (c,j,f) = 128)
        for gp in range(16):
            pe = ps.tile([128, 2, BC], F32, tag="ps")
            for gg_i in range(2):
                g = 2 * gp + gg_i
                nc.tensor.matmul(
                    pe[:, gg_i, :], wpk[:, g, :], gg[:, g, :],
                    start=True, stop=True,
                )
            copy(yy[:, 2 * gp:2 * gp + 2, :], pe[:])

        # regroup2: yh[32f+g, c', i, b] = yy[(c',i,f), g, b].  One piece per
        # (c', i): src = 4 consecutive partitions (iteration f, g, b), dst =
        # all 128 partitions (iteration n, b).  c=0 on the ACT HWDGE ring
        # (emitted after its last E copy, so no head-of-line risk), c=1 on
        # the gpsimd SWDGE ring.
        def r2_piece(c, i):
            eng = nc.sync if (c == 0 and i % 2 == 0) else nc.gpsimd
            p0 = 64 * c + 16 * (i // 4) + (i % 4)
            eng.dma_start(out=yh[:, c, i, :], in_=yy[p0:p0 + 13:4, :, :])

        for i in range(KOUT):
            r2_piece(0, i)
            r2_piece(1, i)

        # keep-warm dummies on the first regroup2 pieces (read free-dim
        # slice i = 4q of the c'=0 piece q from a base-0 partition range)
        for q in range(2):
            pd = psn.tile([128, 512], F32, tag="psn")
            nc.tensor.matmul(
                pd[:],
                dmat[0:16, 0, 0:128],
                yh[0:16, 0, 4 * q, :],
                start=True, stop=True,
            )

        # ---- stage I: inverse DFT, data stationary; chases regroup2
        # i-quad pieces; stores stream per (row-block, i-half).
        for ih in range(2):
            ot = [
                stg.tile([128, 2048], BF16, tag="stg", name=f"ot{ih}_{bs}")
                for bs in range(4)
            ]
            for iqh in range(2):
                iq = 2 * ih + iqh
                for bs in range(4):
                    po = ps.tile([128, 4, 256], F32, tag="ps")
                    for k in range(4):
                        i = 4 * iq + k
                        bsl = yh[:, :, i, 128 * bs:128 * (bs + 1)]
                        nc.tensor.matmul(
                            po[:, k, :], bsl[:, 0, :], dmat[:, 0, :],
                            start=True, stop=False,
                        )
                        nc.tensor.matmul(
                            po[:, k, :], bsl[:, 1, :], dmat[:, 1, :],
                            start=False, stop=True,
                        )
                    copy(ot[bs][:, 1024 * iqh:1024 * (iqh + 1)], po[:])
            for bs in range(4):
                nc.sync.dma_start(
                    out_d[128 * bs:128 * (bs + 1), 2048 * ih:2048 * (ih + 1)],
                    ot[bs][:],
                )

    nc.compile()
    return nc


def _get_program():
    if "nc" not in _CACHE:
        _CACHE["nc"] = _build_program()
    return _CACHE["nc"]


def _install_ntff_hook():
    """Provide antenv.axon_hooks (absent in this image) so that
    run_bass_kernel_spmd(trace=True) can capture NTFF profiles through the
    axon client library."""
    import sys
    import types
    import ctypes
    import contextlib

    if "antenv.axon_hooks" in sys.modules:
        return
    try:
        lib = ctypes.CDLL("/opt/axon/libaxon_pjrt.so")
    except OSError:
        return
    if not hasattr(lib, "axon_start_nrt_profile"):
        return
    lib.axon_start_nrt_profile.argtypes = [
        ctypes.POINTER(ctypes.c_int64),
        ctypes.c_size_t,
    ]
    lib.axon_start_nrt_profile.restype = ctypes.c_int64
    lib.axon_stop_nrt_profile.argtypes = [ctypes.c_char_p]
    lib.axon_stop_nrt_profile.restype = ctypes.c_int64

    @contextlib.contextmanager
    def _hook(output_dir, device_ids):
        import jax

        jax.devices()
        if device_ids:
            ids = (ctypes.c_int64 * len(device_ids))(*device_ids)
            rc = lib.axon_start_nrt_profile(ids, len(device_ids))
        else:
            rc = lib.axon_start_nrt_profile(None, 0)
        if rc != 0:
            raise RuntimeError(f"axon_start_nrt_profile rc={rc}")
        try:
            yield
        finally:
            n = lib.axon_stop_nrt_profile(str(output_dir).encode())
            print(f"ntff profile: {n} file(s) -> {output_dir}")

    mod = types.ModuleType("antenv.axon_hooks")
    state = {"hook": _hook}
    mod.get_axon_ntff_profile_hook = lambda: state["hook"]
    mod.set_axon_ntff_profile_hook = lambda h: state.update(hook=h)
    sys.modules["antenv.axon_hooks"] = mod
    import antenv

    antenv.axon_hooks = mod


def kernel(x, W_real, W_imag, block_size, out_features):
    global LAST_RESULTS
    x = np.asarray(x, dtype=np.float32)
    Wr = np.asarray(W_real, dtype=np.float32)
    Wi = np.asarray(W_imag, dtype=np.float32)
    assert int(block_size) == BS and int(out_features) == D_OUT
    assert x.shape == (B_FULL, D_IN) and Wr.shape == (KOUT, KIN, 129)

    nc = _get_program()
    consts = _build_consts(Wr, Wi)
    core_ids = list(range(NCORES))
    # host-side: cast to bf16 and transpose each core's shard into
    # xt[tl, j, tc, b] = x[b, j*256 + tc*128 + tl]
    xb = x.astype(NPBF16).reshape(NCORES, BC, KIN, 2, 128)
    in_maps = [
        {"x": np.ascontiguousarray(xb[c].transpose(3, 1, 2, 0)), **consts}
        for c in core_ids
    ]
    trace = bool(int(os.environ.get("KERNEL_TRACE", "0")))
    if trace:
        _install_ntff_hook()
    res = run_bass_kernel_spmd(nc, in_maps, core_ids, trace=trace)
    LAST_RESULTS = res
    out = np.concatenate([res.results[c]["out"] for c in core_ids], axis=0)
    return np.ascontiguousarray(out.astype(np.float32))
